# revision 12
# baseline (speedup 1.0000x reference)
"""Trainium2 Bass kernel for nn_MiniAttentionBlock.

Reference computation (B=16, S=4096, F=512):
    h      = tanh(x @ W + b)        [B,S,F]
    scores = h @ u                  [B,S]
    e      = exp(scores)
    a      = e / (sum(e) + eps)     global normalization over all B*S
    out    = sum_s x[b,s,:] * a[b,s]  -> [B,F]

Strategy: data-parallel over batch, 2 batches per core (8 cores).  The
global denominator is one tiny AllReduce of the per-core sum (shipped
replicated on 128 partitions so no broadcast is needed afterwards).
Each core receives its x shard pre-transposed and pre-tiled to
[NSB, 128, NKC, SB] fp16 so every superblock load is one fully
contiguous 1 MB DMA (8 KB per partition line).  x is shipped exactly
ONCE (fp16), serving both the TensorE matmul path and the DVE pooling
path; fp16 keeps the score noise ~8x below bf16.

Per 1024-row superblock (fp16 matmuls, fp32 PSUM):
  - h^T[g, rows] accumulated over 4 k-chunks per g-chunk into a
    [128, 1024] 2-bank PSUM tile (1024-col moving streams amortize the
    per-matmul restart overhead)
  - tanh(+bias) per g-chunk on ScalarE, PSUM -> SBUF fp16
  - scores matmul uses u replicated 128x along the stationary free dim
    so the PSUM result [128, rows] carries scores broadcast to all
    partitions; exp on ScalarE gives e (fp32) with the superblock's
    partial sum accumulated for free (accum_out)
  - weighted pooling sum_rows e*x runs on the DVE as a fused
    multiply + row-reduce custom op
The pooling of the last DEFER superblocks overlaps the AllReduce.
"""

import sys

if "/opt/trn_rl_repo" not in sys.path:
    sys.path.insert(0, "/opt/trn_rl_repo")

import numpy as np

from concourse import bass, bacc, tile, bass_utils
from concourse.dve_ops import TENSOR_TENSOR_REDUCE

mybir = bass.mybir

B, S, F = 16, 4096, 512
N_CORES = 8
BPC = B // N_CORES          # batches per core
R = BPC * S                 # rows per core
SB = 1024                   # rows per superblock
NSB = R // SB               # superblocks per core
DEFER = 3                   # trailing superblocks pooled during the AllReduce
NKC = F // 128              # 128-partition chunks of F
EPS = 1e-7

F32 = mybir.dt.float32
F16 = mybir.dt.float16
ALU = mybir.AluOpType
ACTF = mybir.ActivationFunctionType
AXIS = mybir.AxisListType

_CACHE = {}
_EYE = np.eye(128, dtype=np.float32)


def _build():
    nc = bacc.Bacc("TRN2", target_bir_lowering=False, debug=False,
                   num_devices=N_CORES)

    xh = nc.dram_tensor("xh", [NSB, 128, NKC, SB], F16, kind="ExternalInput")
    w = nc.dram_tensor("w", [F, F], F16, kind="ExternalInput")
    b2 = nc.dram_tensor("b2", [128, NKC], F32, kind="ExternalInput")
    ur = nc.dram_tensor("ur", [128, NKC, 128], F16, kind="ExternalInput")
    eye = nc.dram_tensor("eye", [128, 128], F32, kind="ExternalInput")
    # PE-transposed [BPC*NKC, 128] layout: the final DMA is 8 contiguous
    # 512 B rows instead of 128 32 B partition lines
    out = nc.dram_tensor("out", [BPC * NKC, 128], F32, kind="ExternalOutput")

    with tile.TileContext(nc) as tc:
        with tc.tile_pool(name="const", bufs=1) as cpool, \
             tc.tile_pool(name="xbp", bufs=DEFER + 4) as xbp, \
             tc.tile_pool(name="hap", bufs=8) as hap, \
             tc.tile_pool(name="erp", bufs=DEFER + 2) as erp, \
             tc.tile_pool(name="scr", bufs=4) as scr, \
             tc.tile_pool(name="hps", bufs=2, space="PSUM") as hps, \
             tc.tile_pool(name="sps", bufs=2, space="PSUM") as sps, \
             tc.tile_pool(name="dram", bufs=1, space="DRAM") as dram:

            # warmup collective: pre-warms the ncfw/credit machinery while
            # compute runs; its result is unused
            wu_in = dram.tile([1, 1], F32)
            wu_out = dram.tile([8, 1], F32, addr_space="Shared")
            wu_sb = cpool.tile([1, 1], F32, tag="wusb")
            nc.vector.memset(wu_sb[:], 0.0)
            nc.scalar.dma_start(out=wu_in[:], in_=wu_sb[:])
            nc.gpsimd.collective_compute(
                "AllGather", ALU.bypass,
                replica_groups=[list(range(N_CORES))],
                ins=[wu_in.opt()], outs=[wu_out.opt()])
            ones8 = cpool.tile([8, 128], F32, tag="ones8")
            nc.vector.memset(ones8[:], 1.0)

            # ---- constants ----
            w_sb = []
            for kc in range(NKC):
                t = cpool.tile([128, F], F16, tag=f"w{kc}")
                nc.scalar.dma_start(out=t[:], in_=w.ap()[kc * 128:(kc + 1) * 128, :])
                w_sb.append(t)
            b_sb = cpool.tile([128, NKC], F32, tag="b")
            nc.scalar.dma_start(out=b_sb[:], in_=b2.ap())
            u_sb = cpool.tile([128, NKC, 128], F16, tag="u")
            nc.scalar.dma_start(out=u_sb[:], in_=ur.ap())
            eye_sb = cpool.tile([128, 128], F32, tag="eye")
            nc.scalar.dma_start(out=eye_sb[:], in_=eye.ap())

            esum = cpool.tile([128, NSB], F32, tag="esum")
            nums = [cpool.tile([128, NSB], F32, tag=f"num{kc}", name=f"num{kc}")
                    for kc in range(NKC)]
            out_sb = cpool.tile([128, BPC, NKC], F32, tag="osb")

            # ---- main loop over superblocks ----
            hacts = {}          # sb -> [ha per mc]
            xtiles = {}         # sb -> x sbuf tile
            spsum = {}          # sb -> scores psum tile
            ers = {}            # sb -> exp sbuf tile

            def emit_scores(sb):
                sp = sps.tile([128, SB], F32, tag="s", name="sp")
                has = hacts.pop(sb)
                for half in range(2):
                    cs = slice(half * (SB // 2), (half + 1) * (SB // 2))
                    for mc in range(NKC):
                        nc.tensor.matmul(
                            sp[:, cs],
                            lhsT=u_sb[:, mc, :],
                            rhs=has[mc][:, cs],
                            start=(mc == 0), stop=(mc == NKC - 1))
                spsum[sb] = sp

            def emit_exp(sb):
                sp = spsum.pop(sb)
                er = erp.tile([128, SB], F32, tag="er", name="er")
                nc.scalar.activation(out=er[:], in_=sp[:], func=ACTF.Exp,
                                     accum_out=esum[:, sb:sb + 1])
                ers[sb] = er

            def emit_pool(sb):
                er = ers.pop(sb)
                xall = xtiles.pop(sb)
                for kc in range(NKC):
                    sc = scr.tile([128, SB], F16, tag="sc", name="sc")
                    nc.vector._custom_dve(
                        TENSOR_TENSOR_REDUCE,
                        out=sc[:], in0=xall[:, kc, :], in1=er[:],
                        s0=0.0, s1=1.0,
                        accum_out=nums[kc][:, sb:sb + 1])

            for sb in range(NSB):
                xall = xbp.tile([128, NKC, SB], F16, tag="xb", name="xall")
                if sb <= 1:
                    # fine-grained loads, half-major, so the first matmuls
                    # start as soon as possible while the DMA engines ramp
                    for half in range(2):
                        cs = slice(half * (SB // 2), (half + 1) * (SB // 2))
                        for kc in range(NKC):
                            nc.sync.dma_start(
                                out=xall[:, kc, cs],
                                in_=xh.ap()[sb, :, kc, cs])
                else:
                    # alternate issue queues: descriptor generation costs
                    # ~0.6us per DMA on a sequencer, so spread the issues
                    eng = nc.sync if sb % 2 == 0 else nc.gpsimd
                    eng.dma_start(out=xall[:], in_=xh.ap()[sb])
                xtiles[sb] = xall

                # h^T[g, rows] = sum_f W[f, g] * xT[f, rows]
                # (512-col matmuls: a PSUM bank holds 512 fp32, so each
                # [128, 1024] 2-bank tile is filled as two half sweeps)
                hps_t = [hps.tile([128, SB], F32, tag="h", name="hp")
                         for _ in range(NKC)]
                if sb == 0:
                    # half-major: the first four chunk DMAs feed a full
                    # half-sweep over all mc, halving the startup stall
                    order = [(mc, half) for half in range(2)
                             for mc in range(NKC)]
                else:
                    order = [(mc, half) for mc in range(NKC)
                             for half in range(2)]
                for mc, half in order:
                    cs = slice(half * (SB // 2), (half + 1) * (SB // 2))
                    for kc in range(NKC):
                        nc.tensor.matmul(
                            hps_t[mc][:, cs],
                            lhsT=w_sb[kc][:, mc * 128:(mc + 1) * 128],
                            rhs=xall[:, kc, cs],
                            start=(kc == 0), stop=(kc == NKC - 1))

                # exp of sb-2 FIRST on the Act queue: it releases the
                # scores PSUM slot the PE needs for scores(sb-1)
                if sb >= 2:
                    emit_exp(sb - 2)

                # tanh(+bias), PSUM -> SBUF fp16
                has = []
                for mc in range(NKC):
                    ha = hap.tile([128, SB], F16, tag="h", name="ha")
                    nc.scalar.activation(out=ha[:], in_=hps_t[mc][:],
                                         func=ACTF.Tanh,
                                         bias=b_sb[:, mc:mc + 1])
                    has.append(ha)
                hacts[sb] = has

                # scores of the PREVIOUS superblock (tanh has had time)
                if sb >= 1:
                    emit_scores(sb - 1)
                # pooling of sb-3 after its exp has had time to drain
                if sb >= 3 and (sb - 3) < NSB - DEFER:
                    emit_pool(sb - 3)

            emit_scores(NSB - 1)
            emit_exp(NSB - 2)
            # prefix sum of the denominator while the last exp is in flight
            s_pre = cpool.tile([128, 1], F32, tag="spre")
            nc.vector.tensor_reduce(out=s_pre[:], in_=esum[:, 0:NSB - 1],
                                    axis=AXIS.X, op=ALU.add)
            emit_exp(NSB - 1)

            # ---- global denominator ----
            s_loc = cpool.tile([128, 1], F32, tag="sloc")
            nc.vector.tensor_tensor(out=s_loc[:], in0=s_pre[:],
                                    in1=esum[:, NSB - 1:NSB], op=ALU.add)

            cc_in = dram.tile([1, 1], F32)
            cc_out = dram.tile([8, 1], F32, addr_space="Shared")
            nc.sync.dma_start(out=cc_in[:], in_=s_loc[0:1, 0:1])
            nc.gpsimd.collective_compute(
                "AllGather", ALU.bypass,
                replica_groups=[list(range(N_CORES))],
                ins=[cc_in.opt()], outs=[cc_out.opt()])

            # pooling for the deferred superblocks, overlapping the AllReduce
            for sb in range(NSB - DEFER, NSB):
                emit_pool(sb)

            sg8 = cpool.tile([8, 1], F32, tag="sg8")
            nc.sync.dma_start(out=sg8[:], in_=cc_out[:])

            # per-batch reduction first: depends only on the pooling, so it
            # runs while the AllReduce is still in flight
            for bb in range(BPC):
                for kc in range(NKC):
                    nc.vector.tensor_reduce(
                        out=out_sb[:, bb, kc:kc + 1],
                        in_=nums[kc][:, bb * (NSB // BPC):(bb + 1) * (NSB // BPC)],
                        axis=AXIS.X, op=ALU.add)

            # combine the 8 gathered partials and broadcast to 128 partitions
            # in one small fp32 matmul: ones8^T[128,8] @ sg8[:,0] -> [128,1]
            psg = sps.tile([128, 1], F32, tag="s")
            nc.tensor.matmul(psg[:], lhsT=ones8[:], rhs=sg8[:],
                             start=True, stop=True)
            rcp = cpool.tile([128, 1], F32, tag="rcp")
            nc.vector.tensor_scalar_add(out=rcp[:], in0=psg[:], scalar1=EPS)
            nc.vector.reciprocal(out=rcp[:], in_=rcp[:])
            nc.vector.tensor_scalar_mul(out=out_sb[:], in0=out_sb[:],
                                        scalar1=rcp[:])

            # pack to 8 partitions on the PE, then one row-contiguous DMA
            pt = sps.tile([128, 128], F32, tag="s", name="pt")
            nc.tensor.transpose(
                pt[0:BPC * NKC, :],
                out_sb[:].rearrange("p b c -> p (b c)"),
                eye_sb[:])
            pts = cpool.tile([BPC * NKC, 128], F32, tag="pts")
            nc.scalar.copy(out=pts[:], in_=pt[0:BPC * NKC, :])
            nc.sync.dma_start(out=out.ap(), in_=pts[:])

    nc.compile()
    return nc


def _get_compiled():
    if "nc" not in _CACHE:
        _CACHE["nc"] = _build()
    return _CACHE["nc"]


def _make_in_maps(x, W, b, u):
    Wc = np.ascontiguousarray(np.asarray(W, np.float32).astype(np.float16))
    bc = np.ascontiguousarray(np.asarray(b, np.float32).reshape(NKC, 128).T)
    u_cols = np.asarray(u, np.float32).reshape(NKC, 128).T  # [128, NKC]
    urc = np.ascontiguousarray(
        np.broadcast_to(u_cols[:, :, None], (128, NKC, 128))
    ).astype(np.float16)
    in_maps = []
    for c in range(N_CORES):
        xc = np.asarray(x[BPC * c:BPC * (c + 1)], np.float32).reshape(R, F)
        xt = np.ascontiguousarray(
            xc.T.reshape(NKC, 128, NSB, SB).transpose(2, 1, 0, 3)
        ).astype(np.float16)
        in_maps.append({"xh": xt, "w": Wc, "b2": bc, "ur": urc,
                        "eye": _EYE})
    return in_maps


def kernel(x, W, b, u):
    nc = _get_compiled()
    in_maps = _make_in_maps(x, W, b, u)
    res = bass_utils.run_bass_kernel_spmd(
        nc, in_maps, core_ids=list(range(N_CORES)))
    _CACHE["last_results"] = res
    return np.concatenate(
        [res.results[c]["out"].reshape(BPC, F) for c in range(N_CORES)],
        axis=0)


def kernel_traced(x, W, b, u, **trace_kwargs):
    """Same as kernel() but with NTFF tracing; returns (out, BassKernelResults)."""
    nc = _get_compiled()
    in_maps = _make_in_maps(x, W, b, u)
    res = bass_utils.run_bass_kernel_spmd(
        nc, in_maps, core_ids=list(range(N_CORES)), trace=True, **trace_kwargs)
    _CACHE["last_results"] = res
    out = np.concatenate(
        [res.results[c]["out"].reshape(BPC, F) for c in range(N_CORES)],
        axis=0)
    return out, res


# revision 13
# speedup vs baseline: 1.0804x; 1.0804x over previous
"""Trainium2 Bass kernel for nn_MiniAttentionBlock.

Reference computation (B=16, S=4096, F=512):
    h      = tanh(x @ W + b)        [B,S,F]
    scores = h @ u                  [B,S]
    e      = exp(scores)
    a      = e / (sum(e) + eps)     global normalization over all B*S
    out    = sum_s x[b,s,:] * a[b,s]  -> [B,F]

Strategy: data-parallel over batch, 2 batches per core (8 cores).  The
global denominator is one tiny AllReduce of the per-core sum (shipped
replicated on 128 partitions so no broadcast is needed afterwards).
Each core receives its x shard pre-transposed and pre-tiled to
[NSB, 128, NKC, SB] fp16 so every superblock load is one fully
contiguous 1 MB DMA (8 KB per partition line).  x is shipped exactly
ONCE (fp16), serving both the TensorE matmul path and the DVE pooling
path; fp16 keeps the score noise ~8x below bf16.

Per 1024-row superblock (fp16 matmuls, fp32 PSUM):
  - h^T[g, rows] accumulated over 4 k-chunks per g-chunk into a
    [128, 1024] 2-bank PSUM tile (1024-col moving streams amortize the
    per-matmul restart overhead)
  - tanh(+bias) per g-chunk on ScalarE, PSUM -> SBUF fp16
  - scores matmul uses u replicated 128x along the stationary free dim
    so the PSUM result [128, rows] carries scores broadcast to all
    partitions; exp on ScalarE gives e (fp32) with the superblock's
    partial sum accumulated for free (accum_out)
  - weighted pooling sum_rows e*x runs on the DVE as a fused
    multiply + row-reduce custom op
The pooling of the last DEFER superblocks overlaps the AllReduce.
"""

import sys

if "/opt/trn_rl_repo" not in sys.path:
    sys.path.insert(0, "/opt/trn_rl_repo")

import numpy as np

from concourse import bass, bacc, tile, bass_utils
from concourse.dve_ops import TENSOR_TENSOR_REDUCE

mybir = bass.mybir

B, S, F = 16, 4096, 512
N_CORES = 8
BPC = B // N_CORES          # batches per core
R = BPC * S                 # rows per core
SB = 1024                   # rows per superblock
NSB = R // SB               # superblocks per core
DEFER = 3                   # trailing superblocks pooled during the AllReduce
NKC = F // 128              # 128-partition chunks of F
EPS = 1e-7

F32 = mybir.dt.float32
F16 = mybir.dt.float16
ALU = mybir.AluOpType
ACTF = mybir.ActivationFunctionType
AXIS = mybir.AxisListType

_CACHE = {}
_EYE = np.eye(128, dtype=np.float32)


def _build():
    nc = bacc.Bacc("TRN2", target_bir_lowering=False, debug=False,
                   num_devices=N_CORES)

    xh = nc.dram_tensor("xh", [NSB, 128, NKC, SB], F16, kind="ExternalInput")
    w = nc.dram_tensor("w", [F, F], F16, kind="ExternalInput")
    b2 = nc.dram_tensor("b2", [128, NKC], F32, kind="ExternalInput")
    ur = nc.dram_tensor("ur", [128, NKC, 128], F16, kind="ExternalInput")
    eye = nc.dram_tensor("eye", [128, 128], F32, kind="ExternalInput")
    # PE-transposed [BPC*NKC, 128] layout: the final DMA is 8 contiguous
    # 512 B rows instead of 128 32 B partition lines
    out = nc.dram_tensor("out", [BPC * NKC, 128], F32, kind="ExternalOutput")

    with tile.TileContext(nc) as tc:
        with tc.tile_pool(name="const", bufs=1) as cpool, \
             tc.tile_pool(name="xbp", bufs=DEFER + 4) as xbp, \
             tc.tile_pool(name="hap", bufs=8) as hap, \
             tc.tile_pool(name="erp", bufs=DEFER + 2) as erp, \
             tc.tile_pool(name="scr", bufs=4) as scr, \
             tc.tile_pool(name="hps", bufs=2, space="PSUM") as hps, \
             tc.tile_pool(name="sps", bufs=2, space="PSUM") as sps, \
             tc.tile_pool(name="dram", bufs=1, space="DRAM") as dram:

            # warmup collective: pre-warms the ncfw/credit machinery while
            # compute runs; its result is unused
            wu_in = dram.tile([1, 1], F32)
            wu_out = dram.tile([8, 1], F32, addr_space="Shared")
            wu_sb = cpool.tile([1, 1], F32, tag="wusb")
            nc.vector.memset(wu_sb[:], 0.0)
            nc.scalar.dma_start(out=wu_in[:], in_=wu_sb[:])
            nc.gpsimd.collective_compute(
                "AllGather", ALU.bypass,
                replica_groups=[list(range(N_CORES))],
                ins=[wu_in.opt()], outs=[wu_out.opt()])
            ones8 = cpool.tile([8, 128], F32, tag="ones8")
            nc.vector.memset(ones8[:], 1.0)

            # ---- constants ----
            w_sb = []
            for kc in range(NKC):
                t = cpool.tile([128, F], F16, tag=f"w{kc}")
                nc.scalar.dma_start(out=t[:], in_=w.ap()[kc * 128:(kc + 1) * 128, :])
                w_sb.append(t)
            b_sb = cpool.tile([128, NKC], F32, tag="b")
            nc.scalar.dma_start(out=b_sb[:], in_=b2.ap())
            u_sb = cpool.tile([128, NKC, 128], F16, tag="u")
            nc.scalar.dma_start(out=u_sb[:], in_=ur.ap())
            eye_sb = cpool.tile([128, 128], F32, tag="eye")
            nc.scalar.dma_start(out=eye_sb[:], in_=eye.ap())

            esum = cpool.tile([128, NSB], F32, tag="esum")
            nums = [cpool.tile([128, NSB], F32, tag=f"num{kc}", name=f"num{kc}")
                    for kc in range(NKC)]
            out_sb = cpool.tile([128, BPC, NKC], F32, tag="osb")

            # ---- main loop over superblocks ----
            hacts = {}          # sb -> [ha per mc]
            xtiles = {}         # sb -> x sbuf tile
            spsum = {}          # sb -> scores psum tile
            ers = {}            # sb -> exp sbuf tile

            def emit_scores(sb):
                sp = sps.tile([128, SB], F32, tag="s", name="sp")
                has = hacts.pop(sb)
                for half in range(2):
                    cs = slice(half * (SB // 2), (half + 1) * (SB // 2))
                    for mc in range(NKC):
                        nc.tensor.matmul(
                            sp[:, cs],
                            lhsT=u_sb[:, mc, :],
                            rhs=has[mc][:, cs],
                            start=(mc == 0), stop=(mc == NKC - 1))
                spsum[sb] = sp

            def emit_exp(sb):
                sp = spsum.pop(sb)
                er = erp.tile([128, SB], F32, tag="er", name="er")
                nc.scalar.activation(out=er[:], in_=sp[:], func=ACTF.Exp,
                                     accum_out=esum[:, sb:sb + 1])
                ers[sb] = er

            def emit_pool(sb):
                er = ers.pop(sb)
                xall = xtiles.pop(sb)
                for kc in range(NKC):
                    sc = scr.tile([128, SB], F16, tag="sc", name="sc")
                    nc.vector._custom_dve(
                        TENSOR_TENSOR_REDUCE,
                        out=sc[:], in0=xall[:, kc, :], in1=er[:],
                        s0=0.0, s1=1.0,
                        accum_out=nums[kc][:, sb:sb + 1])

            for sb in range(NSB):
                xall = xbp.tile([128, NKC, SB], F16, tag="xb", name="xall")
                if sb <= 1:
                    # fine-grained loads, half-major, so the first matmuls
                    # start as soon as possible while the DMA engines ramp
                    for half in range(2):
                        cs = slice(half * (SB // 2), (half + 1) * (SB // 2))
                        for kc in range(NKC):
                            nc.sync.dma_start(
                                out=xall[:, kc, cs],
                                in_=xh.ap()[sb, :, kc, cs])
                else:
                    nc.sync.dma_start(out=xall[:], in_=xh.ap()[sb])
                xtiles[sb] = xall

                # h^T[g, rows] = sum_f W[f, g] * xT[f, rows]
                # (512-col matmuls: a PSUM bank holds 512 fp32, so each
                # [128, 1024] 2-bank tile is filled as two half sweeps)
                hps_t = [hps.tile([128, SB], F32, tag="h", name="hp")
                         for _ in range(NKC)]
                if sb == 0:
                    # half-major: the first four chunk DMAs feed a full
                    # half-sweep over all mc, halving the startup stall
                    order = [(mc, half) for half in range(2)
                             for mc in range(NKC)]
                else:
                    order = [(mc, half) for mc in range(NKC)
                             for half in range(2)]
                for mc, half in order:
                    cs = slice(half * (SB // 2), (half + 1) * (SB // 2))
                    for kc in range(NKC):
                        nc.tensor.matmul(
                            hps_t[mc][:, cs],
                            lhsT=w_sb[kc][:, mc * 128:(mc + 1) * 128],
                            rhs=xall[:, kc, cs],
                            start=(kc == 0), stop=(kc == NKC - 1))

                # exp of sb-2 FIRST on the Act queue: it releases the
                # scores PSUM slot the PE needs for scores(sb-1)
                if sb >= 2:
                    emit_exp(sb - 2)

                # tanh(+bias), PSUM -> SBUF fp16
                has = []
                for mc in range(NKC):
                    ha = hap.tile([128, SB], F16, tag="h", name="ha")
                    nc.scalar.activation(out=ha[:], in_=hps_t[mc][:],
                                         func=ACTF.Tanh,
                                         bias=b_sb[:, mc:mc + 1])
                    has.append(ha)
                hacts[sb] = has

                # scores of the PREVIOUS superblock (tanh has had time)
                if sb >= 1:
                    emit_scores(sb - 1)
                # pooling of sb-3 after its exp has had time to drain
                if sb >= 3 and (sb - 3) < NSB - DEFER:
                    emit_pool(sb - 3)

            emit_scores(NSB - 1)
            emit_exp(NSB - 2)
            # prefix sum of the denominator while the last exp is in flight
            s_pre = cpool.tile([128, 1], F32, tag="spre")
            nc.vector.tensor_reduce(out=s_pre[:], in_=esum[:, 0:NSB - 1],
                                    axis=AXIS.X, op=ALU.add)
            emit_exp(NSB - 1)

            # ---- global denominator ----
            s_loc = cpool.tile([128, 1], F32, tag="sloc")
            nc.vector.tensor_tensor(out=s_loc[:], in0=s_pre[:],
                                    in1=esum[:, NSB - 1:NSB], op=ALU.add)

            cc_in = dram.tile([1, 1], F32)
            cc_out = dram.tile([8, 1], F32, addr_space="Shared")
            nc.sync.dma_start(out=cc_in[:], in_=s_loc[0:1, 0:1])
            nc.gpsimd.collective_compute(
                "AllGather", ALU.bypass,
                replica_groups=[list(range(N_CORES))],
                ins=[cc_in.opt()], outs=[cc_out.opt()])

            # pooling for the deferred superblocks, overlapping the AllReduce
            for sb in range(NSB - DEFER, NSB):
                emit_pool(sb)

            sg8 = cpool.tile([8, 1], F32, tag="sg8")
            nc.sync.dma_start(out=sg8[:], in_=cc_out[:])

            # per-batch reduction first: depends only on the pooling, so it
            # runs while the AllReduce is still in flight
            for bb in range(BPC):
                for kc in range(NKC):
                    nc.vector.tensor_reduce(
                        out=out_sb[:, bb, kc:kc + 1],
                        in_=nums[kc][:, bb * (NSB // BPC):(bb + 1) * (NSB // BPC)],
                        axis=AXIS.X, op=ALU.add)

            # combine the 8 gathered partials and broadcast to 128 partitions
            # in one small fp32 matmul: ones8^T[128,8] @ sg8[:,0] -> [128,1]
            psg = sps.tile([128, 1], F32, tag="s")
            nc.tensor.matmul(psg[:], lhsT=ones8[:], rhs=sg8[:],
                             start=True, stop=True)
            rcp = cpool.tile([128, 1], F32, tag="rcp")
            nc.vector.tensor_scalar_add(out=rcp[:], in0=psg[:], scalar1=EPS)
            nc.vector.reciprocal(out=rcp[:], in_=rcp[:])
            nc.vector.tensor_scalar_mul(out=out_sb[:], in0=out_sb[:],
                                        scalar1=rcp[:])

            # pack to 8 partitions on the PE, then one row-contiguous DMA
            pt = sps.tile([128, 128], F32, tag="s", name="pt")
            nc.tensor.transpose(
                pt[0:BPC * NKC, :],
                out_sb[:].rearrange("p b c -> p (b c)"),
                eye_sb[:])
            pts = cpool.tile([BPC * NKC, 128], F32, tag="pts")
            nc.scalar.copy(out=pts[:], in_=pt[0:BPC * NKC, :])
            nc.sync.dma_start(out=out.ap(), in_=pts[:])

    nc.compile()
    return nc


def _get_compiled():
    if "nc" not in _CACHE:
        _CACHE["nc"] = _build()
    return _CACHE["nc"]


def _make_in_maps(x, W, b, u):
    Wc = np.ascontiguousarray(np.asarray(W, np.float32).astype(np.float16))
    bc = np.ascontiguousarray(np.asarray(b, np.float32).reshape(NKC, 128).T)
    u_cols = np.asarray(u, np.float32).reshape(NKC, 128).T  # [128, NKC]
    urc = np.ascontiguousarray(
        np.broadcast_to(u_cols[:, :, None], (128, NKC, 128))
    ).astype(np.float16)
    in_maps = []
    for c in range(N_CORES):
        xc = np.asarray(x[BPC * c:BPC * (c + 1)], np.float32).reshape(R, F)
        xt = np.ascontiguousarray(
            xc.T.reshape(NKC, 128, NSB, SB).transpose(2, 1, 0, 3)
        ).astype(np.float16)
        in_maps.append({"xh": xt, "w": Wc, "b2": bc, "ur": urc,
                        "eye": _EYE})
    return in_maps


def kernel(x, W, b, u):
    nc = _get_compiled()
    in_maps = _make_in_maps(x, W, b, u)
    res = bass_utils.run_bass_kernel_spmd(
        nc, in_maps, core_ids=list(range(N_CORES)))
    _CACHE["last_results"] = res
    return np.concatenate(
        [res.results[c]["out"].reshape(BPC, F) for c in range(N_CORES)],
        axis=0)


def kernel_traced(x, W, b, u, **trace_kwargs):
    """Same as kernel() but with NTFF tracing; returns (out, BassKernelResults)."""
    nc = _get_compiled()
    in_maps = _make_in_maps(x, W, b, u)
    res = bass_utils.run_bass_kernel_spmd(
        nc, in_maps, core_ids=list(range(N_CORES)), trace=True, **trace_kwargs)
    _CACHE["last_results"] = res
    out = np.concatenate(
        [res.results[c]["out"].reshape(BPC, F) for c in range(N_CORES)],
        axis=0)
    return out, res
